# revision 6
# baseline (speedup 1.0000x reference)
"""Multi-head self-attention TRN2 Bass kernel (8-core SPMD).

Problem: x[2,2048,1024] -> qkv proj (w_qkv[1024,3072]) -> 16-head attention
-> out proj (w_out[1024,1024] + b_out) -> [2,2048,1024], all fp32.

Sharding: core i handles batch b=i//4 and head-group g=i%4 (4 heads each).
Each core computes a partial out-projection (its 256 rows of w_out); the
host sums the 4 partials per batch and adds the bias.

Per-core kernel layout strategy:
  - x is transposed on-chip (PE transpose) to xT [c, tokens] once.
  - qT/kT computed per head-pair as [128=2*64 d, 2048 tokens] (tensor-parallel
    over the contraction c in 8 chunks of 128).
  - v computed in natural [keys, d] layout, 4 heads wide (N=256 matmuls),
    stored with a constant-1.0 column appended per head (65-wide blocks) so
    the attention O^T matmul also produces the softmax denominators.
  - Attention per (head, q-slice of 512): St = (k q^T) tiles [128 keys, 512 q]
    in PSUM; ACT exp (scale=1/8 fused) evicts groups of 3 key-chunks to SBUF;
    O^T accumulates over 16 key-chunks with lhsT = v_aug -> [65, 512]
    (rows 0-63 = O^T unnormalized, row 64 = sum of exp).
    Normalize: DVE reciprocal of the sums row, PE broadcast matmul of the
    reciprocal to partitions 64-127 of the same PSUM tile, DVE multiply.
  - Out projection: lhsT = O^T chunks [128 hd, 128 tok], rhs = w_out rows.

Matmuls use float32r (1 cycle/row on TRN2 vs 4 for fp32; ~1e-3 rel precision),
except the broadcast copy which uses fp32.
"""

import sys

if "/opt/trn_rl_repo" not in sys.path:
    sys.path.insert(0, "/opt/trn_rl_repo")

import numpy as np

import concourse.bacc as bacc
import concourse.mybir as mybir
import concourse.tile as tile
from concourse.bass_utils import run_bass_kernel_spmd
from concourse.masks import make_identity

F32 = mybir.dt.float32
F32R = mybir.dt.float32r

N_TOK = 2048
C = 1024
HEADS_PER_CORE = 4
D = 64
CC = C // 128           # 8 contraction chunks
TC = N_TOK // 128       # 16 token chunks
QS = N_TOK // 512       # 4 q-slices
KC = N_TOK // 128       # 16 key chunks
ST_GROUP = 3            # key-chunks per exp group (3 PSUM banks)

_COMPILED = None


def _r(ap):
    return ap.bitcast(F32R)


def build_nc(mm_dt=F32R):
    nc = bacc.Bacc("TRN2", target_bir_lowering=False)

    x_d = nc.declare_dram_parameter("x", [N_TOK, C], F32, isOutput=False)
    wq_d = nc.declare_dram_parameter("wq", [C, 256], F32R, isOutput=False)
    wk_d = nc.declare_dram_parameter("wk", [C, 256], F32R, isOutput=False)
    wv_d = nc.declare_dram_parameter("wv", [C, 256], F32R, isOutput=False)
    wo_d = nc.declare_dram_parameter("wo", [256, C], F32R, isOutput=False)
    out_d = nc.declare_dram_parameter("out", [N_TOK, C], F32, isOutput=True)

    cast = lambda ap: ap

    with tile.TileContext(nc) as tc:
        with (
            tc.tile_pool(name="const", bufs=1) as const_pool,
            tc.tile_pool(name="qkt", bufs=1) as qkt_pool,
            tc.tile_pool(name="vsb", bufs=1) as v_pool,
            tc.tile_pool(name="otsb", bufs=1) as ot_sb_pool,
            tc.tile_pool(name="wo", bufs=1) as wo_pool,
            tc.tile_pool(name="outsb", bufs=2) as out_sb_pool,
        ):
            identity = const_pool.tile([128, 128], F32, tag="ident")
            make_identity(nc, identity[:])
            ones_col = const_pool.tile([1, 64], F32, tag="ones")
            nc.vector.memset(ones_col[:], 1.0)

            # persistent per-pair tensors
            qT = [qkt_pool.tile([128, N_TOK], F32R, tag=f"qT{p}", name=f"qT{p}") for p in range(2)]
            kT = [qkt_pool.tile([128, N_TOK], F32R, tag=f"kT{p}", name=f"kT{p}") for p in range(2)]
            oT = [ot_sb_pool.tile([128, N_TOK], F32R, tag=f"oT{p}", name=f"oT{p}") for p in range(2)]
            v_sb = v_pool.tile([128, KC, 4 * 65], F32R, tag="v")
            # ones column per 65-wide head block
            v_aug_view = v_sb[:].rearrange("p k (h e) -> p k h e", h=4)
            ones64 = const_pool.tile([128, 64], F32, tag="ones64")
            nc.vector.memset(ones64[:], 1.0)
            nc.vector.tensor_copy(
                v_aug_view[:, :, :, 64:65],
                ones64[:].rearrange("p (k h e) -> p k h e", k=KC, h=4),
            )

            wo_sb = wo_pool.tile([128, 2, C], F32R, tag="wo")
            nc.sync.dma_start(
                out=wo_sb[:], in_=wo_d[:].rearrange("(cc p) n -> p cc n", p=128)
            )

            # ---- phase 1: load x, transpose to xT; compute qT/kT/v ----
            with (
                tc.tile_pool(name="xT", bufs=1) as xt_pool,
                tc.tile_pool(name="xnat", bufs=3) as xnat_pool,
                tc.tile_pool(name="w3", bufs=3) as w3_pool,
                tc.tile_pool(name="tpp", bufs=4, space="PSUM") as tp_psum,
                tc.tile_pool(name="qkvp", bufs=4, space="PSUM") as qkv_psum,
            ):
                wq_sb = w3_pool.tile([128, CC, 256], F32R, tag="w3")
                wk_sb = w3_pool.tile([128, CC, 256], F32R, tag="w3")
                wv_sb = w3_pool.tile([128, CC, 256], F32R, tag="w3")
                for w_d, w_sb in ((wq_d, wq_sb), (wk_d, wk_sb), (wv_d, wv_sb)):
                    nc.sync.dma_start(
                        out=w_sb[:], in_=w_d[:].rearrange("(cc p) n -> p cc n", p=128)
                    )

                xT = xt_pool.tile([128, CC, N_TOK], F32R, tag="xT")
                for t in range(TC):
                    x_nat = xnat_pool.tile([128, C], F32, tag="xnat")
                    nc.sync.dma_start(out=x_nat[:], in_=x_d[t * 128:(t + 1) * 128, :])
                    for cc in range(CC):
                        tp = tp_psum.tile([128, 128], F32, tag="tp")
                        nc.tensor.transpose(
                            tp[:], x_nat[:, cc * 128:(cc + 1) * 128], identity[:]
                        )
                        nc.vector.tensor_copy(
                            xT[:, cc, t * 128:(t + 1) * 128], tp[:]
                        )

                # qT / kT per pair, accumulated over c-chunks
                for ts in range(QS):
                    sl = slice(ts * 512, (ts + 1) * 512)
                    for p in range(2):
                        for w_sb, dst in ((wq_sb, qT[p]), (wk_sb, kT[p])):
                            acc = qkv_psum.tile([128, 512], F32, tag="qkv")
                            for cc in range(CC):
                                nc.tensor.matmul(
                                    acc[:],
                                    cast(w_sb[:, cc, p * 128:(p + 1) * 128]),
                                    cast(xT[:, cc, sl]),
                                    start=(cc == 0),
                                    stop=(cc == CC - 1),
                                )
                            nc.vector.tensor_copy(dst[:, sl], acc[:])

                # v natural [keys, d], all 4 heads (N=256)
                for kc in range(KC):
                    acc = qkv_psum.tile([128, 256], F32, tag="qkv")
                    for cc in range(CC):
                        nc.tensor.matmul(
                            acc[:],
                            cast(xT[:, cc, kc * 128:(kc + 1) * 128]),
                            cast(wv_sb[:, cc, :]),
                            start=(cc == 0),
                            stop=(cc == CC - 1),
                        )
                    nc.vector.tensor_copy(
                        v_aug_view[:, kc, :, 0:64],
                        acc[:].rearrange("p (h e) -> p h e", h=4),
                    )

            # ---- phase 2: attention + out projection ----
            n_groups = (KC + ST_GROUP - 1) // ST_GROUP
            with (
                tc.tile_pool(name="stp", bufs=2, space="PSUM") as st_psum,
                tc.tile_pool(name="otp", bufs=2, space="PSUM") as ot_psum,
                tc.tile_pool(name="pt", bufs=3) as pt_pool,
                tc.tile_pool(name="rcp", bufs=2) as rcp_pool,
            ):
                for ts in range(QS):
                    sl = slice(ts * 512, (ts + 1) * 512)
                    for p in range(2):
                        for h in range(2):
                            a = 2 * p + h
                            hp = slice(h * 64, (h + 1) * 64)
                            ot = ot_psum.tile([128, 512], F32, tag="ot")
                            for g in range(n_groups):
                                gsz = min(ST_GROUP, KC - g * ST_GROUP)
                                st = st_psum.tile([128, ST_GROUP * 512], F32, tag="st")
                                pt = pt_pool.tile([128, ST_GROUP * 512], F32R, tag="pt")
                                for j in range(gsz):
                                    kc = g * ST_GROUP + j
                                    nc.tensor.matmul(
                                        st[:, j * 512:(j + 1) * 512],
                                        cast(kT[p][hp, kc * 128:(kc + 1) * 128]),
                                        cast(qT[p][hp, sl]),
                                        start=True,
                                        stop=True,
                                    )
                                nc.scalar.activation(
                                    pt[:, : gsz * 512],
                                    st[:, : gsz * 512],
                                    mybir.ActivationFunctionType.Exp,
                                    scale=0.125,
                                )
                                for j in range(gsz):
                                    kc = g * ST_GROUP + j
                                    nc.tensor.matmul(
                                        ot[0:65, :],
                                        cast(v_sb[:, kc, a * 65:(a + 1) * 65]),
                                        cast(pt[:, j * 512:(j + 1) * 512]),
                                        start=(kc == 0),
                                        stop=(kc == KC - 1),
                                    )
                            rcp = rcp_pool.tile([1, 512], F32, tag="rcp")
                            nc.vector.reciprocal(rcp[:], ot[64:65, :])
                            # broadcast reciprocal to partitions 64..127 (fp32)
                            nc.tensor.matmul(
                                ot[64:128, :],
                                ones_col[0:1, :],
                                rcp[0:1, :],
                                start=True,
                                stop=True,
                                tile_position=(0, 64),
                            )
                            nc.vector.tensor_copy(oT[p][hp, sl], ot[0:64, :])
                            nc.vector.tensor_mul(
                                oT[p][hp, sl], oT[p][hp, sl], ot[64:128, :]
                            )
                    # out projection for the 4 token-chunks of this q-slice
                    for t in range(4 * ts, 4 * ts + 4):
                        outp = out_sb_pool.tile([128, C], F32, tag="outp")
                        for ns in range(2):
                            po = ot_psum.tile([128, 512], F32, tag="ot")
                            for p in range(2):
                                nc.tensor.matmul(
                                    po[:],
                                    cast(oT[p][:, t * 128:(t + 1) * 128]),
                                    cast(wo_sb[:, p, ns * 512:(ns + 1) * 512]),
                                    start=(p == 0),
                                    stop=(p == 1),
                                )
                            nc.vector.tensor_copy(outp[:, ns * 512:(ns + 1) * 512], po[:])
                        nc.sync.dma_start(
                            out=out_d[t * 128:(t + 1) * 128, :], in_=outp[:]
                        )

    nc.compile()
    return nc


def _shard_inputs(x, w_qkv, w_out):
    in_maps = []
    for i in range(8):
        b, g = divmod(i, 4)
        cs = slice(256 * g, 256 * (g + 1))
        in_maps.append({
            "x": np.ascontiguousarray(x[b]),
            "wq": np.ascontiguousarray(w_qkv[:, cs]),
            "wk": np.ascontiguousarray(w_qkv[:, 1024 + 256 * g:1024 + 256 * (g + 1)]),
            "wv": np.ascontiguousarray(w_qkv[:, 2048 + 256 * g:2048 + 256 * (g + 1)]),
            "wo": np.ascontiguousarray(w_out[cs, :]),
        })
    return in_maps


def kernel(x, w_qkv, w_out, b_out):
    global _COMPILED
    x = np.asarray(x, np.float32)
    w_qkv = np.asarray(w_qkv, np.float32)
    w_out = np.asarray(w_out, np.float32)
    b_out = np.asarray(b_out, np.float32)

    if _COMPILED is None:
        _COMPILED = build_nc()
    nc = _COMPILED

    in_maps = _shard_inputs(x, w_qkv, w_out)
    res = run_bass_kernel_spmd(nc, in_maps, core_ids=list(range(8)))
    out = np.zeros((2, N_TOK, C), np.float32)
    for i in range(8):
        b = i // 4
        out[b] += res.results[i]["out"]
    out += b_out[None, None, :]
    return out


# revision 7
# speedup vs baseline: 1.1533x; 1.1533x over previous
"""Multi-head self-attention TRN2 Bass kernel (8-core SPMD).

Problem: x[2,2048,1024] -> qkv proj (w_qkv[1024,3072]) -> 16-head attention
-> out proj (w_out[1024,1024] + b_out) -> [2,2048,1024], all fp32.

Sharding: core i handles batch b=i//4 and head-group g=i%4 (4 heads each).
Each core computes a partial out-projection (its 256 rows of w_out); the
host sums the 4 partials per batch and adds the bias.

Per-core kernel layout strategy:
  - x is transposed on-chip (PE transpose) to xT [c, tokens] once.
  - qT/kT computed per head-pair as [128=2*64 d, 2048 tokens] (tensor-parallel
    over the contraction c in 8 chunks of 128).
  - v computed in natural [keys, d] layout, 4 heads wide (N=256 matmuls),
    stored with a constant-1.0 column appended per head (65-wide blocks) so
    the attention O^T matmul also produces the softmax denominators.
  - Attention per (head, q-slice of 512): St = (k q^T) tiles [128 keys, 512 q]
    in PSUM; ACT exp (scale=1/8 fused) evicts groups of 3 key-chunks to SBUF;
    O^T accumulates over 16 key-chunks with lhsT = v_aug -> [65, 512]
    (rows 0-63 = O^T unnormalized, row 64 = sum of exp).
    Normalize: DVE reciprocal of the sums row, PE broadcast matmul of the
    reciprocal to partitions 64-127 of the same PSUM tile, DVE multiply.
  - Out projection: lhsT = O^T chunks [128 hd, 128 tok], rhs = w_out rows.

Matmuls use float32r (1 cycle/row on TRN2 vs 4 for fp32; ~1e-3 rel precision),
except the broadcast copy which uses fp32.
"""

import sys

if "/opt/trn_rl_repo" not in sys.path:
    sys.path.insert(0, "/opt/trn_rl_repo")

import numpy as np

import concourse.bacc as bacc
import concourse.mybir as mybir
import concourse.tile as tile
from concourse.bass_utils import run_bass_kernel_spmd
from concourse.masks import make_identity

F32 = mybir.dt.float32
F32R = mybir.dt.float32r

N_TOK = 2048
C = 1024
HEADS_PER_CORE = 4
D = 64
CC = C // 128           # 8 contraction chunks
TC = N_TOK // 128       # 16 token chunks
QS = N_TOK // 512       # 4 q-slices
KC = N_TOK // 128       # 16 key chunks
ST_GROUP = 3            # key-chunks per exp group (3 PSUM banks)

_COMPILED = None


def _r(ap):
    return ap.bitcast(F32R)


def build_nc(mm_dt=F32R):
    nc = bacc.Bacc("TRN2", target_bir_lowering=False)

    x_d = nc.declare_dram_parameter("x", [N_TOK, C], F32, isOutput=False)
    wq_d = nc.declare_dram_parameter("wq", [C, 256], F32R, isOutput=False)
    wk_d = nc.declare_dram_parameter("wk", [C, 256], F32R, isOutput=False)
    wv_d = nc.declare_dram_parameter("wv", [C, 256], F32R, isOutput=False)
    wo_d = nc.declare_dram_parameter("wo", [256, C], F32R, isOutput=False)
    out_d = nc.declare_dram_parameter("out", [N_TOK, C], F32, isOutput=True)

    with tile.TileContext(nc) as tc:
        with (
            tc.tile_pool(name="const", bufs=1) as const_pool,
            tc.tile_pool(name="qkt", bufs=1) as qkt_pool,
            tc.tile_pool(name="vsb", bufs=1) as v_pool,
            tc.tile_pool(name="otsb", bufs=1) as ot_sb_pool,
            tc.tile_pool(name="wo", bufs=1) as wo_pool,
            tc.tile_pool(name="outsb", bufs=2) as out_sb_pool,
            tc.tile_pool(name="pss", bufs=2, space="PSUM") as ps_small,
        ):
            identity = const_pool.tile([128, 128], F32, tag="ident")
            make_identity(nc, identity[:])

            # persistent per-pair tensors
            qT = [qkt_pool.tile([128, N_TOK], F32R, tag=f"qT{p}", name=f"qT{p}") for p in range(2)]
            kT = [qkt_pool.tile([128, N_TOK], F32R, tag=f"kT{p}", name=f"kT{p}") for p in range(2)]
            oT = [ot_sb_pool.tile([128, N_TOK], F32R, tag=f"oT{p}", name=f"oT{p}") for p in range(2)]
            # v_aug: per key-chunk, per head: [64 v columns | 64 ones columns]
            # so the O^T matmul (M=128) also emits the softmax denominators,
            # already broadcast across partitions 64..127.
            v_sb = v_pool.tile([128, KC, 4 * 128], F32R, tag="v")
            v_aug_view = v_sb[:].rearrange("p k (h e) -> p k h e", h=4)
            ones256 = const_pool.tile([128, 256], F32, tag="ones256")
            nc.vector.memset(ones256[:], 1.0)
            for kc in range(KC):
                nc.vector.tensor_copy(
                    v_aug_view[:, kc, :, 64:128],
                    ones256[:].rearrange("p (h e) -> p h e", h=4),
                )

            wo_sb = wo_pool.tile([128, 2, C], F32R, tag="wo")
            nc.sync.dma_start(
                out=wo_sb[:], in_=wo_d[:].rearrange("(cc p) n -> p cc n", p=128)
            )

            # ---- phase 1: load x, transpose to xT; compute qT/kT/v ----
            with (
                tc.tile_pool(name="xT", bufs=1) as xt_pool,
                tc.tile_pool(name="xnat", bufs=3) as xnat_pool,
                tc.tile_pool(name="w3", bufs=3) as w3_pool,
            ):
                wq_sb = w3_pool.tile([128, CC, 256], F32R, tag="w3")
                wk_sb = w3_pool.tile([128, CC, 256], F32R, tag="w3")
                wv_sb = w3_pool.tile([128, CC, 256], F32R, tag="w3")
                for w_d, w_sb in ((wq_d, wq_sb), (wk_d, wk_sb), (wv_d, wv_sb)):
                    nc.sync.dma_start(
                        out=w_sb[:], in_=w_d[:].rearrange("(cc p) n -> p cc n", p=128)
                    )

                xT = xt_pool.tile([128, CC, N_TOK], F32R, tag="xT")
                with tc.tile_pool(name="tpp", bufs=2, space="PSUM") as tp_psum:
                    for t in range(TC):
                        x_nat = xnat_pool.tile([128, C], F32, tag="xnat")
                        nc.sync.dma_start(out=x_nat[:], in_=x_d[t * 128:(t + 1) * 128, :])
                        tp = tp_psum.tile([128, CC, 128], F32, tag="tp")
                        for cc in range(CC):
                            nc.tensor.transpose(
                                tp[:, cc, :], x_nat[:, cc * 128:(cc + 1) * 128], identity[:]
                            )
                        nc.vector.tensor_copy(xT[:, :, t * 128:(t + 1) * 128], tp[:])

                # qT / kT, pair-major so attention on pair 0 can start early
                for p in range(2):
                    for ts in range(QS):
                        sl = slice(ts * 512, (ts + 1) * 512)
                        for w_sb, dst in ((wq_sb, qT[p]), (wk_sb, kT[p])):
                            acc = ps_small.tile([128, 512], F32, tag="ps")
                            for cc in range(CC):
                                nc.tensor.matmul(
                                    acc[:],
                                    w_sb[:, cc, p * 128:(p + 1) * 128],
                                    xT[:, cc, sl],
                                    start=(cc == 0),
                                    stop=(cc == CC - 1),
                                )
                            nc.vector.tensor_copy(dst[:, sl], acc[:])

                # v natural [keys, d], all 4 heads (N=256)
                for kc in range(KC):
                    acc = ps_small.tile([128, 256], F32, tag="ps")
                    for cc in range(CC):
                        nc.tensor.matmul(
                            acc[:],
                            xT[:, cc, kc * 128:(kc + 1) * 128],
                            wv_sb[:, cc, :],
                            start=(cc == 0),
                            stop=(cc == CC - 1),
                        )
                    nc.vector.tensor_copy(
                        v_aug_view[:, kc, :, 0:64],
                        acc[:].rearrange("p (h e) -> p h e", h=4),
                    )

            # ---- phase 2: attention + out projection ----
            n_groups = (KC + ST_GROUP - 1) // ST_GROUP
            with (
                tc.tile_pool(name="stp", bufs=2, space="PSUM") as st_psum,
                tc.tile_pool(name="pt", bufs=4) as pt_pool,
                tc.tile_pool(name="rcp", bufs=4) as rcp_pool,
            ):
                for ts in range(QS):
                    sl = slice(ts * 512, (ts + 1) * 512)
                    for p in range(2):
                        # both heads of the pair interleaved: S matmuls land on
                        # PE row-groups (0,0)/(64,0) and can run concurrently.
                        ots = [ps_small.tile([128, 512], F32, tag="ps", name=f"ot{h}") for h in range(2)]
                        for g in range(n_groups):
                            gsz = min(ST_GROUP, KC - g * ST_GROUP)
                            for h in range(2):
                                hp = slice(h * 64, (h + 1) * 64)
                                st = st_psum.tile([128, ST_GROUP * 512], F32, tag="st", name=f"st{h}")
                                pt = pt_pool.tile([128, ST_GROUP * 512], F32R, tag="pt", name=f"pt{h}")
                                for j in range(gsz):
                                    kc = g * ST_GROUP + j
                                    nc.tensor.matmul(
                                        st[:, j * 512:(j + 1) * 512],
                                        kT[p][hp, kc * 128:(kc + 1) * 128],
                                        qT[p][hp, sl],
                                        start=True,
                                        stop=True,
                                    )
                                nc.scalar.activation(
                                    pt[:, : gsz * 512],
                                    st[:, : gsz * 512],
                                    mybir.ActivationFunctionType.Exp,
                                    scale=0.125,
                                )
                                a = 2 * p + h
                                for j in range(gsz):
                                    kc = g * ST_GROUP + j
                                    nc.tensor.matmul(
                                        ots[h][:, :],
                                        v_sb[:, kc, a * 128:(a + 1) * 128],
                                        pt[:, j * 512:(j + 1) * 512],
                                        start=(kc == 0),
                                        stop=(kc == KC - 1),
                                    )
                        for h in range(2):
                            hp = slice(h * 64, (h + 1) * 64)
                            rcp = rcp_pool.tile([64, 512], F32, tag="rcp")
                            nc.vector.reciprocal(rcp[:], ots[h][64:128, :])
                            nc.vector.tensor_mul(oT[p][hp, sl], ots[h][0:64, :], rcp[:])
                    # out projection for the 4 token-chunks of this q-slice
                    for t in range(4 * ts, 4 * ts + 4):
                        outp = out_sb_pool.tile([128, C], F32, tag="outp")
                        for ns in range(2):
                            po = ps_small.tile([128, 512], F32, tag="ps")
                            for p in range(2):
                                nc.tensor.matmul(
                                    po[:],
                                    oT[p][:, t * 128:(t + 1) * 128],
                                    wo_sb[:, p, ns * 512:(ns + 1) * 512],
                                    start=(p == 0),
                                    stop=(p == 1),
                                )
                            nc.vector.tensor_copy(outp[:, ns * 512:(ns + 1) * 512], po[:])
                        nc.sync.dma_start(
                            out=out_d[t * 128:(t + 1) * 128, :], in_=outp[:]
                        )

    nc.compile()
    return nc


def _shard_inputs(x, w_qkv, w_out):
    in_maps = []
    for i in range(8):
        b, g = divmod(i, 4)
        cs = slice(256 * g, 256 * (g + 1))
        in_maps.append({
            "x": np.ascontiguousarray(x[b]),
            "wq": np.ascontiguousarray(w_qkv[:, cs]),
            "wk": np.ascontiguousarray(w_qkv[:, 1024 + 256 * g:1024 + 256 * (g + 1)]),
            "wv": np.ascontiguousarray(w_qkv[:, 2048 + 256 * g:2048 + 256 * (g + 1)]),
            "wo": np.ascontiguousarray(w_out[cs, :]),
        })
    return in_maps


def kernel(x, w_qkv, w_out, b_out):
    global _COMPILED
    x = np.asarray(x, np.float32)
    w_qkv = np.asarray(w_qkv, np.float32)
    w_out = np.asarray(w_out, np.float32)
    b_out = np.asarray(b_out, np.float32)

    if _COMPILED is None:
        _COMPILED = build_nc()
    nc = _COMPILED

    in_maps = _shard_inputs(x, w_qkv, w_out)
    res = run_bass_kernel_spmd(nc, in_maps, core_ids=list(range(8)))
    out = np.zeros((2, N_TOK, C), np.float32)
    for i in range(8):
        b = i // 4
        out[b] += res.results[i]["out"]
    out += b_out[None, None, :]
    return out
